# revision 7
# baseline (speedup 1.0000x reference)
"""Trainium2 Bass kernel for the scatter_memory echo-state-network module.

Full computation (see reference):
    lr    = softmax_u((X @ adaptive_lr) / temperature)          [B,U,1]
    feed  = X @ Win          (per-unit)                          [B,U,N]
    echo  = state @ (W*sr) + bias                                [B,U,N]
    ns    = (1-lr)*state + lr*tanh(feed+echo)                    [B,U,N]
    out   = ns @ Wout                                            [B,U,O]

Sharding: units axis, 2 units per core across 8 cores. The softmax over
units is replicated on every core (each core gets the full X transposed +
adaptive_lr and computes all 16 logits itself — no collectives).

Numerics: weights stream in fp16 (DMA-bound kernel → halves traffic).
Win/bias are exactly representable (0/1 valued). X is split into fp16
hi+lo so feed = X@Win stays fp32-accurate; logits are computed in fp32.
PSUM accumulation is fp32 throughout.
"""

import numpy as np

import concourse.bass as bass
from concourse import bacc
import concourse.mybir as mybir
import concourse.tile as tile
from concourse.bass_utils import run_bass_kernel_spmd

f32 = mybir.dt.float32
f16 = mybir.dt.float16
AF = mybir.ActivationFunctionType
ALU = mybir.AluOpType

B, U, N, D, O = 64, 16, 2048, 1024, 1024
NCORES = 8
UPC = U // NCORES          # units per core
P = 128
KC_D = D // P              # 8 k-chunks for D-contraction
KC_N = N // P              # 16 k-chunks for N-contraction
NT = N // 512              # 4 n-tiles of 512
OT = O // 512              # 2 o-tiles of 512

_prog_cache = {}


def build_program():
    nc = bacc.Bacc("TRN2", target_bir_lowering=False, debug=False,
                   num_devices=NCORES)

    # [p, (u, kc, b)] fp32 — X transposed, all units (for logits + local feed)
    xt32_d = nc.dram_tensor("xt32", [P, U * KC_D * B], f32, kind="ExternalInput")
    # [p, (ui, kc, b)] fp16 hi/lo split of local units' XT
    xt16h_d = nc.dram_tensor("xt16h", [P, UPC * KC_D * B], f16, kind="ExternalInput")
    xt16l_d = nc.dram_tensor("xt16l", [P, UPC * KC_D * B], f16, kind="ExternalInput")
    # [p, (ui, kc, b)] fp16 — stateT pre-scaled by sr[u]
    st16_d = nc.dram_tensor("st16", [P, UPC * KC_N * B], f16, kind="ExternalInput")
    # [b, (ui, n)] fp32 — state natural layout (for the update)
    state32_d = nc.dram_tensor("state32", [B, UPC * N], f32, kind="ExternalInput")
    # [p, (u, kc)] fp32 — adaptive_lr / temperature
    alr32_d = nc.dram_tensor("alr32", [P, U * KC_D], f32, kind="ExternalInput")
    # [1, (ui, n)] fp16
    bias16_d = nc.dram_tensor("bias16", [1, UPC * N], f16, kind="ExternalInput")
    # weight streams, contiguous per (ui, kc) tile
    w16_d = nc.dram_tensor("w16", [UPC, KC_N, P, N], f16, kind="ExternalInput")
    win16_d = nc.dram_tensor("win16", [UPC, KC_D, P, N], f16, kind="ExternalInput")
    wout16_d = nc.dram_tensor("wout16", [UPC, KC_N, P, O], f16, kind="ExternalInput")

    ns_d = nc.dram_tensor("ns_out", [B, UPC * N], f32, kind="ExternalOutput")
    o_d = nc.dram_tensor("o_out", [B, UPC * O], f32, kind="ExternalOutput")

    ident_d = nc.inline_tensor(np.eye(B, dtype=np.float32), name="ident64")
    ones_d = nc.inline_tensor(np.ones((1, B), dtype=np.float16), name="ones64")

    with tile.TileContext(nc) as tc:
        with (
            tc.tile_pool(name="res", bufs=1) as res,
            tc.tile_pool(name="wstream", bufs=4) as wpool,
            tc.tile_pool(name="wostream", bufs=3) as wopool,
            tc.tile_pool(name="work", bufs=3) as work,
            tc.tile_pool(name="pse", bufs=4, space="PSUM") as pse,
            tc.tile_pool(name="pst", bufs=2, space="PSUM") as pst,
            tc.tile_pool(name="pso", bufs=2, space="PSUM") as pso,
        ):
            # ---- resident loads (feed operands first: PE can start earliest) ----
            xt16h_s = res.tile([P, UPC * KC_D * B], f16, tag="xt16h")
            nc.sync.dma_start(xt16h_s[:], xt16h_d[:])
            xt16l_s = res.tile([P, UPC * KC_D * B], f16, tag="xt16l")
            nc.sync.dma_start(xt16l_s[:], xt16l_d[:])
            st16_s = res.tile([P, UPC * KC_N * B], f16, tag="st16")
            nc.sync.dma_start(st16_s[:], st16_d[:])
            alr_s = res.tile([P, U * KC_D], f32, tag="alr")
            nc.sync.dma_start(alr_s[:], alr32_d[:])
            xt32_s = res.tile([P, U * KC_D * B], f32, tag="xt32")
            nc.sync.dma_start(xt32_s[:], xt32_d[:])
            state32_s = res.tile([B, UPC * N], f32, tag="state32")
            nc.sync.dma_start(state32_s[:], state32_d[:])
            bias_s = res.tile([1, UPC * N], f16, tag="bias")
            nc.sync.dma_start(bias_s[:], bias16_d[:])
            ident_s = res.tile([B, B], f32, tag="ident")
            nc.sync.dma_start(ident_s[:], ident_d[:])
            ones_s = res.tile([1, B], f16, tag="ones")
            nc.sync.dma_start(ones_s[:], ones_d[:])
            nsT16_s = res.tile([P, UPC * KC_N * B], f16, tag="nsT")
            lr_s = res.tile([B, U], f32, tag="lr")

            # ---- logits: [64, 16] fp32, one accumulation group per unit column ----
            logit_ps = pst.tile([B, U], f32, tag="tr")
            for u in range(U):
                for kc in range(KC_D):
                    nc.tensor.matmul(
                        logit_ps[:, u : u + 1],
                        lhsT=xt32_s[:, (u * KC_D + kc) * B : (u * KC_D + kc + 1) * B],
                        rhs=alr_s[:, u * KC_D + kc : u * KC_D + kc + 1],
                        start=(kc == 0),
                        stop=(kc == KC_D - 1),
                    )

            # ---- softmax over units (free axis) ----
            mx = work.tile([B, 1], f32, tag="sm")
            nc.vector.reduce_max(mx, logit_ps, axis=mybir.AxisListType.X)
            neg_mx = work.tile([B, 1], f32, tag="sm2")
            nc.vector.tensor_scalar_mul(neg_mx, mx, -1.0)
            ex_s = work.tile([B, U], f32, tag="sm3")
            nc.scalar.activation(ex_s, logit_ps, AF.Exp, bias=neg_mx, scale=1.0)
            sm = work.tile([B, 1], f32, tag="sm4")
            nc.vector.reduce_sum(sm, ex_s, axis=mybir.AxisListType.X)
            rs = work.tile([B, 1], f32, tag="sm5")
            nc.vector.reciprocal(rs, sm)
            nc.vector.tensor_scalar_mul(lr_s, ex_s, rs)

            # ---- per-unit heavy pipeline ----
            for ui in range(UPC):
                # feed + echo + bias accumulate into 4 psum tiles (one per n-tile)
                ps_e = [pse.tile([B, 512], f32, tag="e", name=f"pse{ui}_{i}") for i in range(NT)]
                for kc in range(KC_D):
                    win_t = wpool.tile([P, N], f16, tag="w")
                    nc.sync.dma_start(win_t, win16_d[ui, kc])
                    xh = xt16h_s[:, (ui * KC_D + kc) * B : (ui * KC_D + kc + 1) * B]
                    xl = xt16l_s[:, (ui * KC_D + kc) * B : (ui * KC_D + kc + 1) * B]
                    for nt in range(NT):
                        rhs = win_t[:, nt * 512 : (nt + 1) * 512]
                        nc.tensor.matmul(ps_e[nt], lhsT=xh, rhs=rhs,
                                         start=(kc == 0), stop=False)
                        nc.tensor.matmul(ps_e[nt], lhsT=xl, rhs=rhs,
                                         start=False, stop=False)
                for kc in range(KC_N):
                    w_t = wpool.tile([P, N], f16, tag="w")
                    nc.sync.dma_start(w_t, w16_d[ui, kc])
                    st = st16_s[:, (ui * KC_N + kc) * B : (ui * KC_N + kc + 1) * B]
                    for nt in range(NT):
                        nc.tensor.matmul(ps_e[nt], lhsT=st,
                                         rhs=w_t[:, nt * 512 : (nt + 1) * 512],
                                         start=False, stop=False)
                for nt in range(NT):
                    nc.tensor.matmul(ps_e[nt], lhsT=ones_s,
                                     rhs=bias_s[:, ui * N + nt * 512 : ui * N + (nt + 1) * 512],
                                     start=False, stop=True)

                # tanh + state update + writeback + transpose for the out matmul
                for nt in range(NT):
                    ssl = state32_s[:, ui * N + nt * 512 : ui * N + (nt + 1) * 512]
                    t = work.tile([B, 512], f32, tag="tanh")
                    nc.scalar.activation(t, ps_e[nt], AF.Tanh)
                    nc.vector.tensor_tensor(out=t, in0=t, in1=ssl, op=ALU.subtract)
                    nc.vector.tensor_scalar(
                        out=t, in0=t,
                        scalar1=lr_s[:, ui : ui + 1], scalar2=None, op0=ALU.mult,
                    )
                    ns_t = work.tile([B, 512], f32, tag="ns")
                    nc.vector.tensor_tensor(out=ns_t, in0=t, in1=ssl, op=ALU.add)
                    nc.sync.dma_start(
                        ns_d[:, ui * N + nt * 512 : ui * N + (nt + 1) * 512], ns_t)
                    for j in range(4):
                        kc = nt * 4 + j
                        pt = pst.tile([P, B], f32, tag="tr")
                        nc.tensor.transpose(
                            pt, ns_t[:, j * P : (j + 1) * P], ident_s)
                        nc.vector.tensor_copy(
                            nsT16_s[:, (ui * KC_N + kc) * B : (ui * KC_N + kc + 1) * B],
                            pt)

                # out = ns @ Wout
                ps_o = [pso.tile([B, 512], f32, tag="o", name=f"pso{ui}_{i}") for i in range(OT)]
                for kc in range(KC_N):
                    wo_t = wopool.tile([P, O], f16, tag="wo")
                    nc.sync.dma_start(wo_t, wout16_d[ui, kc])
                    nsT = nsT16_s[:, (ui * KC_N + kc) * B : (ui * KC_N + kc + 1) * B]
                    for ot in range(OT):
                        nc.tensor.matmul(ps_o[ot], lhsT=nsT,
                                         rhs=wo_t[:, ot * 512 : (ot + 1) * 512],
                                         start=(kc == 0), stop=(kc == KC_N - 1))
                for ot in range(OT):
                    o_t = work.tile([B, 512], f32, tag="o_s")
                    nc.scalar.copy(o_t, ps_o[ot])
                    nc.sync.dma_start(
                        o_d[:, ui * O + ot * 512 : ui * O + (ot + 1) * 512], o_t)

    nc.compile()
    return nc


def prep_core_inputs(core, X, state, W, Win, bias, Wout, alr_eff, sr):
    """Build the in_map for one core (units 2*core, 2*core+1). All numpy."""
    u0 = core * UPC
    # Unit order rotated so this core's own units sit in logit columns 0..UPC-1
    # (the kernel reads lr at local column ui; softmax is order-invariant).
    order = [(u0 + j) % U for j in range(U)]
    # X transposed for all units: [p, u, kc, b]
    xt = X.transpose(2, 1, 0).reshape(KC_D, P, U, B).transpose(1, 2, 0, 3)
    xt32 = np.ascontiguousarray(
        xt[:, order].reshape(P, U * KC_D * B), dtype=np.float32)
    # local-unit hi/lo split
    xloc = xt[:, u0 : u0 + UPC]                       # [p, UPC, kc, b] f32
    xh = xloc.astype(np.float16)
    xl = (xloc - xh.astype(np.float32)).astype(np.float16)
    xt16h = np.ascontiguousarray(xh.reshape(P, UPC * KC_D * B))
    xt16l = np.ascontiguousarray(xl.reshape(P, UPC * KC_D * B))
    # stateT scaled by sr: [p, ui, kc, b]
    sloc = state[:, u0 : u0 + UPC, :] * sr[u0 : u0 + UPC, 0, 0][None, :, None]
    stt = sloc.transpose(2, 1, 0).reshape(KC_N, P, UPC, B).transpose(1, 2, 0, 3)
    st16 = np.ascontiguousarray(stt.reshape(P, UPC * KC_N * B), dtype=np.float16)
    state32 = np.ascontiguousarray(
        state[:, u0 : u0 + UPC, :].reshape(B, UPC * N), dtype=np.float32)
    alr32 = np.ascontiguousarray(
        alr_eff[:, :, 0].T.reshape(KC_D, P, U).transpose(1, 2, 0)[:, order]
        .reshape(P, U * KC_D),
        dtype=np.float32)
    bias16 = np.ascontiguousarray(
        bias[u0 : u0 + UPC, 0, :].reshape(1, UPC * N), dtype=np.float16)
    w16 = np.ascontiguousarray(
        W[u0 : u0 + UPC].reshape(UPC, KC_N, P, N), dtype=np.float16)
    win16 = np.ascontiguousarray(
        Win[u0 : u0 + UPC].reshape(UPC, KC_D, P, N), dtype=np.float16)
    wout16 = np.ascontiguousarray(
        Wout[u0 : u0 + UPC].reshape(UPC, KC_N, P, O), dtype=np.float16)
    return {
        "xt32": xt32, "xt16h": xt16h, "xt16l": xt16l, "st16": st16,
        "state32": state32, "alr32": alr32, "bias16": bias16,
        "w16": w16, "win16": win16, "wout16": wout16,
    }


def kernel(X, state, W, Win, bias, Wout, adaptive_lr, sr, temperature, **run_kwargs):
    X = np.asarray(X, dtype=np.float32)
    state = np.asarray(state, dtype=np.float32)
    W = np.asarray(W, dtype=np.float32)
    Win = np.asarray(Win, dtype=np.float32)
    bias = np.asarray(bias, dtype=np.float32)
    Wout = np.asarray(Wout, dtype=np.float32)
    adaptive_lr = np.asarray(adaptive_lr, dtype=np.float32)
    sr = np.asarray(sr, dtype=np.float32)
    alr_eff = adaptive_lr / np.float32(np.asarray(temperature).reshape(-1)[0])

    if "nc" not in _prog_cache:
        _prog_cache["nc"] = build_program()
    nc = _prog_cache["nc"]

    in_maps = [
        prep_core_inputs(c, X, state, W, Win, bias, Wout, alr_eff, sr)
        for c in range(NCORES)
    ]
    res = run_bass_kernel_spmd(nc, in_maps, core_ids=list(range(NCORES)), **run_kwargs)

    new_state = np.empty((B, U, N), dtype=np.float32)
    output = np.empty((B, U, O), dtype=np.float32)
    for c in range(NCORES):
        u0 = c * UPC
        new_state[:, u0 : u0 + UPC, :] = res.results[c]["ns_out"].reshape(B, UPC, N)
        output[:, u0 : u0 + UPC, :] = res.results[c]["o_out"].reshape(B, UPC, O)
    if run_kwargs:
        return (new_state, output), res
    return new_state, output
